# revision 6
# baseline (speedup 1.0000x reference)
"""Trainium2 Bass kernel for nn_ASAP (gnn_message_passing), 8 NeuronCores.

Sharding strategy (edge-parallel, dst-bucketed):
  - Edges are sorted by destination node and bucketed into 8 equal node
    ranges (12500 nodes/core), so each core owns the complete segment-max
    for its node range and no cross-core max reduction is needed.
  - Each core's edge shard is materialized as per-edge records
    [pos_src, pos_dst, x_dst] (the edge-parallel shard of node features),
    shipped in a feature-major layout the TensorEngine consumes directly.
  - Each node's edge list is padded to a multiple of D=4 slots ("virtual
    nodes"), so the per-node max becomes a fixed periodic-4 reduction;
    per-node combining of virtual nodes happens at unshard time.
  - Global scalars (direction norm, per-layer BatchNorm statistics) are
    computed on device and combined with AllReduce collectives.

Device pipeline (single NEFF, SPMD on 8 cores):
  phase A: stream [posS,posD] records, accumulate ||posS-posD||^2 -> AllReduce
  pass 1:  L1 matmul (dir/s fold in weights) + relu evict + stats1 -> AllReduce
  pass 2:  L1 again, L2 matmul (BN1 folded) + bias/pad-kill matmul + relu
           evict + stats2, h2 -> DRAM -> AllReduce
  pass 3:  h2 from DRAM, L3 matmul (BN2 folded) + bias/kill + relu evict +
           stats3 + periodic-4 max -> vnode buffer -> AllReduce
  pass 4:  BN3 affine (monotone, commutes with max) applied to vnode maxes.
"""

import sys

sys.path.insert(0, "/opt/trn_rl_repo")

import numpy as np
import ml_dtypes

N_NODES = 100000
HIDDEN = 64
IN_CH = 6
BN_EPS = 1e-5
N_CORES = 8
NODES_PER_CORE = N_NODES // N_CORES  # 12500
D = 4          # virtual-node slot granularity
CHUNK = 512    # slots per matmul chunk (free dim)
PAIR = 2 * CHUNK

BF16 = ml_dtypes.bfloat16

_compiled = None  # (nc, npairs) cache


def _ceil_to(x, m):
    return (x + m - 1) // m * m


def _host_shard(x, pos, edge_index):
    """Sort edges by dst, bucket by node range, build per-core slot arrays."""
    src = np.asarray(edge_index[0], dtype=np.int64)
    dst = np.asarray(edge_index[1], dtype=np.int64)
    E = src.shape[0]
    order = np.argsort(dst, kind="stable")
    src_s, dst_s = src[order], dst[order]
    # core boundaries by node range
    bounds = np.searchsorted(dst_s, np.arange(0, N_NODES + 1, NODES_PER_CORE))
    pos32 = np.asarray(pos, dtype=np.float32)
    x32 = np.asarray(x, dtype=np.float32)

    shards = []
    max_slots = 0
    for k in range(N_CORES):
        lo, hi = bounds[k], bounds[k + 1]
        s_k, d_k = src_s[lo:hi], dst_s[lo:hi] - k * NODES_PER_CORE
        deg = np.bincount(d_k, minlength=NODES_PER_CORE)
        pdeg = _ceil_to(deg, D)
        nslots = int(pdeg.sum())
        starts = np.concatenate([[0], np.cumsum(pdeg)[:-1]])
        estarts = np.concatenate([[0], np.cumsum(deg)[:-1]])
        rank = np.arange(hi - lo) - np.repeat(estarts, deg)
        slot = np.repeat(starts, deg) + rank
        shards.append((s_k, d_k, deg, pdeg, nslots, slot))
        max_slots = max(max_slots, nslots)
    S_pad = _ceil_to(max_slots, 2 * PAIR)
    npairs = S_pad // PAIR

    in_maps = []
    unshard = []
    for k in range(N_CORES):
        s_k, d_k, deg, pdeg, nslots, slot = shards[k]
        rec = np.zeros((S_pad, 16), dtype=np.float32)
        rec[slot, 0:3] = pos32[s_k]
        rec[slot, 3:6] = pos32[d_k + k * NODES_PER_CORE]
        rec[slot, 6:9] = x32[d_k + k * NODES_PER_CORE]
        valid = np.zeros(S_pad, dtype=np.float32)
        valid[slot] = 1.0
        # feature-major, two 512-slot chunks stacked on partitions
        feat2 = (
            rec.reshape(npairs, 2, CHUNK, 16)
            .transpose(1, 3, 0, 2)
            .reshape(2 * 16, npairs * CHUNK)
            .astype(BF16)
        )
        # edge-major dir records [S_pad, 8] = [posS(3), posD(3), 0, 0]
        dirrec = np.zeros((S_pad, 8), dtype=np.float32)
        dirrec[:, 0:6] = rec[:, 0:6]
        dirrec = dirrec.astype(BF16)
        # flag rows [4, npairs*CHUNK]: realA, padA, realB, padB
        v2 = valid.reshape(npairs, 2, CHUNK)
        flags = np.zeros((4, npairs, CHUNK), dtype=np.float32)
        flags[0] = v2[:, 0]
        flags[1] = 1.0 - v2[:, 0]
        flags[2] = v2[:, 1]
        flags[3] = 1.0 - v2[:, 1]
        flags = flags.reshape(4, npairs * CHUNK).astype(BF16)
        in_maps.append({"feat2": feat2, "dirrec": dirrec, "flags": flags})
        unshard.append((deg, pdeg, nslots))
    return in_maps, unshard, S_pad, npairs, E


def _build(npairs, E_total):
    import concourse.bacc as bacc
    import concourse.bass as bass
    from concourse import mybir
    from concourse.tile import TileContext

    F32 = mybir.dt.float32
    B16 = mybir.dt.bfloat16
    AX = mybir.AxisListType
    OP = mybir.AluOpType
    ACTF = mybir.ActivationFunctionType
    S2 = npairs * CHUNK  # columns of feat2 (= slots/2)
    NVC = npairs * 128   # vnode columns of output per partition-half

    nc = bacc.Bacc("TRN2", target_bir_lowering=False, debug=True)

    feat2 = nc.declare_dram_parameter("feat2", [32, S2], B16, isOutput=False)
    dirrec = nc.declare_dram_parameter("dirrec", [S2 * 2, 8], B16, isOutput=False)
    flags = nc.declare_dram_parameter("flags", [4, S2], B16, isOutput=False)
    w1dir = nc.declare_dram_parameter("w1dir", [3, HIDDEN], F32, isOutput=False)
    w1x = nc.declare_dram_parameter("w1x", [3, HIDDEN], F32, isOutput=False)
    w2 = nc.declare_dram_parameter("w2", [HIDDEN, HIDDEN], F32, isOutput=False)
    w3 = nc.declare_dram_parameter("w3", [HIDDEN, HIDDEN], F32, isOutput=False)
    # gvec rows: 0:g1 1:be1 2:b1 3:g2 4:be2 5:b2 6:g3 7:be3 8:b3  as [9, 64]
    gvec = nc.declare_dram_parameter("gvec", [9, HIDDEN], F32, isOutput=False)
    out = nc.declare_dram_parameter("out", [128, NVC], F32, isOutput=True)

    h2store = nc.dram_tensor("h2store", [128, S2], B16)
    vnstore = nc.dram_tensor("vnstore", [128, NVC], F32)
    cc_in = nc.dram_tensor("cc_in", [128, 4], F32)
    cc_out = nc.dram_tensor("cc_out", [128, 4], F32, addr_space="Shared")
    RG = [list(range(N_CORES))]

    with TileContext(nc) as tc:
        with (
            tc.tile_pool(name="const", bufs=1) as cp,
            tc.tile_pool(name="stream", bufs=4) as sp,
            tc.tile_pool(name="hbuf", bufs=4) as hp,
            tc.tile_pool(name="acc", bufs=1) as ap,
            tc.tile_pool(name="psum", bufs=2, space="PSUM") as pp,
        ):
            # ---- constants / small tiles ----
            w1d_t = cp.tile([3, HIDDEN], F32)
            nc.sync.dma_start(out=w1d_t[:], in_=w1dir[:])
            w1x_t = cp.tile([3, HIDDEN], F32)
            nc.sync.dma_start(out=w1x_t[:], in_=w1x[:])
            w2_t = cp.tile([HIDDEN, HIDDEN], F32)
            nc.sync.dma_start(out=w2_t[:], in_=w2[:])
            w3_t = cp.tile([HIDDEN, HIDDEN], F32)
            nc.sync.dma_start(out=w3_t[:], in_=w3[:])
            gv_t = cp.tile([9, HIDDEN], F32)
            nc.sync.dma_start(out=gv_t[:], in_=gvec[:])

            # ---- phase A: global direction norm ----
            dacc = ap.tile([128, 1], F32)
            nc.gpsimd.memset(dacc[:], 0.0)
            ntiles_a = (2 * S2) // 4096
            for t in range(ntiles_a):
                dt_ = sp.tile([128, 32, 8], B16, tag="dirt")
                nc.sync.dma_start(
                    out=dt_[:],
                    in_=dirrec[:].rearrange("(a p b) f -> a p b f", p=128, b=32)[t],
                )
                dsub = hp.tile([128, 32, 3], F32, tag="dsub")
                nc.vector.tensor_tensor(
                    out=dsub[:], in0=dt_[:, :, 0:3], in1=dt_[:, :, 3:6], op=OP.subtract
                )
                dsq = hp.tile([128, 32, 3], F32, tag="dsq")
                nc.vector.tensor_tensor(
                    out=dsq[:], in0=dsub[:], in1=dsub[:], op=OP.mult
                )
                dred = hp.tile([128, 1], F32, tag="dred")
                nc.vector.tensor_reduce(
                    out=dred[:], in_=dsq[:].rearrange("p a b -> p (a b)"),
                    axis=AX.X, op=OP.add,
                )
                nc.vector.tensor_tensor(
                    out=dacc[:], in0=dacc[:], in1=dred[:], op=OP.add
                )
            # partition-reduce to scalar then AllReduce
            dtot = ap.tile([1, 1], F32)
            nc.gpsimd.tensor_reduce(
                out=dtot[:], in_=dacc[:], axis=AX.C, op=OP.add
            )
            cc_s = ap.tile([128, 4], F32)
            nc.gpsimd.memset(cc_s[:], 0.0)
            nc.vector.tensor_copy(out=cc_s[0:1, 0:1], in_=dtot[:])
            nc.sync.dma_start(out=cc_in[:], in_=cc_s[:])
            nc.gpsimd.collective_compute(
                "AllReduce", OP.add, replica_groups=RG,
                ins=[cc_in[:]], outs=[cc_out[:]],
            )
            nsq = ap.tile([1, 1], F32)
            nc.sync.dma_start(out=nsq[:], in_=cc_out[0:1, 0:1])
            # rs = 1/||dir|| = reciprocal(sqrt(sum))
            nrm = ap.tile([1, 1], F32)
            nc.scalar.activation(out=nrm[:], in_=nsq[:], func=ACTF.Sqrt)
            rs1 = ap.tile([1, 1], F32)
            nc.vector.reciprocal(out=rs1[:], in_=nrm[:])
            rs128 = ap.tile([128, 1], F32)
            nc.gpsimd.partition_broadcast(rs128[:], rs1[:])

            # ---- build L1 weights: lhsT1 [32, 128] bf16 ----
            lhsT1 = cp.tile([32, 128], B16)
            nc.gpsimd.memset(lhsT1[:], 0.0)
            w1d_p = cp.tile([3, HIDDEN], B16)
            nc.vector.tensor_scalar(
                out=w1d_p[:], in0=w1d_t[:], scalar1=rs128[0:3, 0:1],
                scalar2=None, op0=OP.mult,
            )
            w1d_n = cp.tile([3, HIDDEN], B16)
            nc.vector.tensor_scalar(
                out=w1d_n[:], in0=w1d_t[:], scalar1=rs128[0:3, 0:1],
                scalar2=-1.0, op0=OP.mult, op1=OP.mult,
            )
            w1x_b = cp.tile([3, HIDDEN], B16)
            nc.vector.tensor_copy(out=w1x_b[:], in_=w1x_t[:])
            for ko, base in ((0, 0), (16, 64)):
                nc.sync.dma_start(out=lhsT1[ko + 0:ko + 3, base:base + 64], in_=w1d_p[:])
                nc.sync.dma_start(out=lhsT1[ko + 3:ko + 6, base:base + 64], in_=w1d_n[:])
                nc.sync.dma_start(out=lhsT1[ko + 6:ko + 9, base:base + 64], in_=w1x_b[:])

            # ---- helpers ----
            def stats_sync(sumbuf, sqbuf, dst_stats):
                """Reduce per-chunk stat columns, AllReduce, return mu/var tiles."""
                sred = ap.tile([128, 1], F32, tag="sred" + dst_stats)
                nc.vector.tensor_reduce(
                    out=sred[:], in_=sumbuf[:], axis=AX.X, op=OP.add
                )
                qred = ap.tile([128, 1], F32, tag="qred" + dst_stats)
                nc.vector.tensor_reduce(
                    out=qred[:], in_=sqbuf[:], axis=AX.X, op=OP.add
                )
                cc2 = ap.tile([128, 4], F32, tag="cc2" + dst_stats)
                nc.gpsimd.memset(cc2[:], 0.0)
                hi2 = ap.tile([64, 2], F32, tag="hi2" + dst_stats)
                nc.sync.dma_start(out=hi2[:, 0:1], in_=sred[64:128, :])
                nc.sync.dma_start(out=hi2[:, 1:2], in_=qred[64:128, :])
                nc.vector.tensor_tensor(
                    out=cc2[0:64, 0:1], in0=sred[0:64, :], in1=hi2[:, 0:1], op=OP.add
                )
                nc.vector.tensor_tensor(
                    out=cc2[0:64, 1:2], in0=qred[0:64, :], in1=hi2[:, 1:2], op=OP.add
                )
                nc.sync.dma_start(out=cc_in[:], in_=cc2[:])
                nc.gpsimd.collective_compute(
                    "AllReduce", OP.add, replica_groups=RG,
                    ins=[cc_in[:]], outs=[cc_out[:]],
                )
                gl = ap.tile([64, 2], F32, tag="gl" + dst_stats)
                nc.sync.dma_start(out=gl[:], in_=cc_out[0:64, 0:2])
                invE = 1.0 / float(E_total)
                mu = ap.tile([64, 1], F32, tag="mu" + dst_stats)
                nc.vector.tensor_scalar(
                    out=mu[:], in0=gl[:, 0:1], scalar1=invE, scalar2=None, op0=OP.mult
                )
                musq = ap.tile([64, 1], F32, tag="musq" + dst_stats)
                nc.vector.tensor_tensor(out=musq[:], in0=mu[:], in1=mu[:], op=OP.mult)
                var = ap.tile([64, 1], F32, tag="var" + dst_stats)
                nc.vector.tensor_scalar(
                    out=var[:], in0=gl[:, 1:2], scalar1=invE, scalar2=None, op0=OP.mult
                )
                nc.vector.tensor_tensor(
                    out=var[:], in0=var[:], in1=musq[:], op=OP.subtract
                )
                return mu, var

            def fold_affine(mu, var, g_row, be_row, tag):
                """s = g*rsqrt(var+eps); t = be - mu*s  (per-channel [64,1])."""
                vps = ap.tile([64, 1], F32, tag="vps" + tag)
                nc.vector.tensor_scalar(
                    out=vps[:], in0=var[:], scalar1=BN_EPS, scalar2=None, op0=OP.add
                )
                sd = ap.tile([64, 1], F32, tag="sd" + tag)
                nc.scalar.activation(out=sd[:], in_=vps[:], func=ACTF.Sqrt)
                rsd = ap.tile([64, 1], F32, tag="rsd" + tag)
                nc.vector.reciprocal(out=rsd[:], in_=sd[:])
                # g, be arrive as rows [1,64] of gvec; transpose to col via PE? cheap:
                # use dma transpose-free: gvec rows are [1,64]; we need [64,1].
                # DMA from DRAM with AP [64,1] view of the row.
                gcol = ap.tile([64, 1], F32, tag="gcol" + tag)
                nc.sync.dma_start(out=gcol[:], in_=gvec[g_row, :, None])
                becol = ap.tile([64, 1], F32, tag="becol" + tag)
                nc.sync.dma_start(out=becol[:], in_=gvec[be_row, :, None])
                s_ = ap.tile([64, 1], F32, tag="s" + tag)
                nc.vector.tensor_tensor(out=s_[:], in0=gcol[:], in1=rsd[:], op=OP.mult)
                t_ = ap.tile([64, 1], F32, tag="t" + tag)
                nc.vector.tensor_tensor(out=t_[:], in0=mu[:], in1=s_[:], op=OP.mult)
                nc.vector.tensor_tensor(
                    out=t_[:], in0=becol[:], in1=t_[:], op=OP.subtract
                )
                return s_, t_

            def build_layer(mu, var, g_row, be_row, b_row, w_t, tag):
                """lhsT [128,128] = blockdiag(diag(s)W, diag(s)W) bf16,
                killT [4,128] rows: [beff@A, -BIG@A, beff@B, -BIG@B]."""
                s_, t_ = fold_affine(mu, var, g_row, be_row, tag)
                lhsT = cp.tile([128, 128], B16, tag="lhsT" + tag)
                nc.gpsimd.memset(lhsT[:], 0.0)
                nc.vector.tensor_scalar(
                    out=lhsT[0:64, 0:64], in0=w_t[:], scalar1=s_[:], scalar2=None,
                    op0=OP.mult,
                )
                nc.vector.tensor_scalar(
                    out=lhsT[64:128, 64:128], in0=w_t[:], scalar1=s_[:], scalar2=None,
                    op0=OP.mult,
                )
                # beff row = t^T @ W + b : matmul lhsT=t(col) rhs=W -> [1,64]
                tb = ap.tile([64, 1], B16, tag="tb" + tag)
                nc.vector.tensor_copy(out=tb[:], in_=t_[:])
                wb = ap.tile([64, 64], B16, tag="wb" + tag)
                nc.vector.tensor_copy(out=wb[:], in_=w_t[:])
                bp = pp.tile([1, 64], F32, tag="bp", space="PSUM")
                nc.tensor.matmul(out=bp[:], lhsT=tb[:], rhs=wb[:])
                brow = ap.tile([1, 64], F32, tag="brow" + tag)
                nc.vector.tensor_copy(out=brow[:], in_=bp[:])
                bcol = ap.tile([64, 1], F32, tag="bcol" + tag)
                nc.sync.dma_start(out=bcol[:], in_=gvec[b_row, :, None])
                # add layer bias b (zeros in practice): brow += b^T — b is [64,1]; do
                # via dma-transposed view add
                btr = ap.tile([1, 64], F32, tag="btr" + tag)
                nc.sync.dma_start(out=btr[:], in_=gvec[b_row, None, :])
                nc.vector.tensor_tensor(
                    out=brow[:], in0=brow[:], in1=btr[:], op=OP.add
                )
                killT = cp.tile([4, 128], B16, tag="killT" + tag)
                nc.gpsimd.memset(killT[:], 0.0)
                negbig = ap.tile([1, 64], B16, tag="negbig" + tag)
                nc.gpsimd.memset(negbig[:], -1e30)
                browb = ap.tile([1, 64], B16, tag="browb" + tag)
                nc.vector.tensor_copy(out=browb[:], in_=brow[:])
                nc.sync.dma_start(out=killT[0:1, 0:64], in_=browb[:])
                nc.sync.dma_start(out=killT[2:3, 64:128], in_=browb[:])
                nc.sync.dma_start(out=killT[1:2, 0:64], in_=negbig[:])
                nc.sync.dma_start(out=killT[3:4, 64:128], in_=negbig[:])
                return lhsT, killT, s_, t_

            # ---- pass 1: L1 + stats1 ----
            sum1 = ap.tile([128, npairs], F32)
            sq1 = ap.tile([128, npairs], F32)
            for c in range(npairs):
                rhs = sp.tile([32, CHUNK], B16, tag="rhs1")
                nc.sync.dma_start(out=rhs[:], in_=feat2[:, c * CHUNK:(c + 1) * CHUNK])
                z1 = pp.tile([128, CHUNK], F32, tag="zA", space="PSUM")
                nc.tensor.matmul(out=z1[:], lhsT=lhsT1[:], rhs=rhs[:])
                h1 = hp.tile([128, CHUNK], B16, tag="h1")
                nc.scalar.activation(
                    out=h1[:], in_=z1[:], func=ACTF.Relu, bias=0.0, scale=1.0,
                    accum_out=sum1[:, c:c + 1],
                )
                hsq = hp.tile([128, CHUNK], B16, tag="hsq")
                nc.vector.tensor_tensor(out=hsq[:], in0=h1[:], in1=h1[:], op=OP.mult)
                nc.vector.tensor_reduce(
                    out=sq1[:, c:c + 1], in_=hsq[:], axis=AX.X, op=OP.add
                )
            mu1, var1 = stats_sync(sum1, sq1, "1")
            lhsT2, killT2, _, _ = build_layer(mu1, var1, 0, 1, 2, w2_t, "2")

            # ---- pass 2: L1 + L2 + stats2, h2 -> DRAM ----
            sum2 = ap.tile([128, npairs], F32)
            sq2 = ap.tile([128, npairs], F32)
            for c in range(npairs):
                rhs = sp.tile([32, CHUNK], B16, tag="rhs2")
                nc.sync.dma_start(out=rhs[:], in_=feat2[:, c * CHUNK:(c + 1) * CHUNK])
                z1 = pp.tile([128, CHUNK], F32, tag="zA", space="PSUM")
                nc.tensor.matmul(out=z1[:], lhsT=lhsT1[:], rhs=rhs[:])
                h1 = hp.tile([128, CHUNK], B16, tag="h1b")
                nc.scalar.activation(
                    out=h1[:], in_=z1[:], func=ACTF.Relu, bias=0.0, scale=1.0
                )
                fl = sp.tile([4, CHUNK], B16, tag="fl2")
                nc.sync.dma_start(out=fl[:], in_=flags[:, c * CHUNK:(c + 1) * CHUNK])
                z2 = pp.tile([128, CHUNK], F32, tag="zB", space="PSUM")
                nc.tensor.matmul(out=z2[:], lhsT=lhsT2[:], rhs=h1[:], start=True, stop=False)
                nc.tensor.matmul(out=z2[:], lhsT=killT2[:], rhs=fl[:], start=False, stop=True)
                h2 = hp.tile([128, CHUNK], B16, tag="h2")
                nc.scalar.activation(
                    out=h2[:], in_=z2[:], func=ACTF.Relu, bias=0.0, scale=1.0,
                    accum_out=sum2[:, c:c + 1],
                )
                nc.sync.dma_start(
                    out=h2store[:, c * CHUNK:(c + 1) * CHUNK], in_=h2[:]
                )
                hsq = hp.tile([128, CHUNK], B16, tag="hsq2")
                nc.vector.tensor_tensor(out=hsq[:], in0=h2[:], in1=h2[:], op=OP.mult)
                nc.vector.tensor_reduce(
                    out=sq2[:, c:c + 1], in_=hsq[:], axis=AX.X, op=OP.add
                )
            mu2, var2 = stats_sync(sum2, sq2, "2")
            lhsT3, killT3, _, _ = build_layer(mu2, var2, 3, 4, 5, w3_t, "3")

            # ---- pass 3: L3 + stats3 + periodic-4 max ----
            sum3 = ap.tile([128, npairs], F32)
            sq3 = ap.tile([128, npairs], F32)
            for c in range(npairs):
                h2 = sp.tile([128, CHUNK], B16, tag="h2r")
                nc.sync.dma_start(
                    out=h2[:], in_=h2store[:, c * CHUNK:(c + 1) * CHUNK]
                )
                fl = sp.tile([4, CHUNK], B16, tag="fl3")
                nc.sync.dma_start(out=fl[:], in_=flags[:, c * CHUNK:(c + 1) * CHUNK])
                z3 = pp.tile([128, CHUNK], F32, tag="zB", space="PSUM")
                nc.tensor.matmul(out=z3[:], lhsT=lhsT3[:], rhs=h2[:], start=True, stop=False)
                nc.tensor.matmul(out=z3[:], lhsT=killT3[:], rhs=fl[:], start=False, stop=True)
                h3 = hp.tile([128, CHUNK], B16, tag="h3")
                nc.scalar.activation(
                    out=h3[:], in_=z3[:], func=ACTF.Relu, bias=0.0, scale=1.0,
                    accum_out=sum3[:, c:c + 1],
                )
                hsq = hp.tile([128, CHUNK], B16, tag="hsq3")
                nc.vector.tensor_tensor(out=hsq[:], in0=h3[:], in1=h3[:], op=OP.mult)
                nc.vector.tensor_reduce(
                    out=sq3[:, c:c + 1], in_=hsq[:], axis=AX.X, op=OP.add
                )
                vmax = hp.tile([128, 128], F32, tag="vmax")
                nc.vector.tensor_reduce(
                    out=vmax[:], in_=h3[:].rearrange("p (a b) -> p a b", b=D),
                    axis=AX.X, op=OP.max,
                )
                nc.sync.dma_start(
                    out=vnstore[:, c * 128:(c + 1) * 128], in_=vmax[:]
                )
            mu3, var3 = stats_sync(sum3, sq3, "4")
            s3, t3 = fold_affine(mu3, var3, 6, 7, "5")
            s3d = ap.tile([128, 1], F32)
            nc.sync.dma_start(out=s3d[0:64, :], in_=s3[:])
            nc.sync.dma_start(out=s3d[64:128, :], in_=s3[:])
            t3d = ap.tile([128, 1], F32)
            nc.sync.dma_start(out=t3d[0:64, :], in_=t3[:])
            nc.sync.dma_start(out=t3d[64:128, :], in_=t3[:])

            # ---- pass 4: BN3 affine on vnode maxes ----
            VW = 2048
            for c in range((NVC + VW - 1) // VW):
                w = min(VW, NVC - c * VW)
                vt = sp.tile([128, VW], F32, tag="vt")
                nc.sync.dma_start(
                    out=vt[:, 0:w], in_=vnstore[:, c * VW:c * VW + w]
                )
                ot = hp.tile([128, VW], F32, tag="ot")
                nc.vector.tensor_scalar(
                    out=ot[:, 0:w], in0=vt[:, 0:w], scalar1=s3d[:], scalar2=t3d[:],
                    op0=OP.mult, op1=OP.add,
                )
                nc.sync.dma_start(out=out[:, c * VW:c * VW + w], in_=ot[:, 0:w])

    nc.compile()
    return nc


def kernel(x, pos, edge_index, W1, b1, g1, be1, W2, b2, g2, be2, W3, b3, g3, be3):
    global _compiled
    from concourse.bass_utils import run_bass_kernel_spmd

    in_maps, unshard, S_pad, npairs, E = _host_shard(x, pos, edge_index)

    W1f = np.asarray(W1, dtype=np.float32)
    gv = np.stack([
        np.asarray(g1, dtype=np.float32), np.asarray(be1, dtype=np.float32),
        np.asarray(b1, dtype=np.float32), np.asarray(g2, dtype=np.float32),
        np.asarray(be2, dtype=np.float32), np.asarray(b2, dtype=np.float32),
        np.asarray(g3, dtype=np.float32), np.asarray(be3, dtype=np.float32),
        np.asarray(b3, dtype=np.float32),
    ])
    common = {
        "w1dir": W1f[0:3], "w1x": W1f[3:6],
        "w2": np.asarray(W2, dtype=np.float32),
        "w3": np.asarray(W3, dtype=np.float32),
        "gvec": gv,
    }
    for m in in_maps:
        m.update(common)

    if _compiled is None or _compiled[1] != npairs:
        _compiled = (_build(npairs, E), npairs)
    nc = _compiled[0]

    res = run_bass_kernel_spmd(nc, in_maps, core_ids=list(range(N_CORES)))

    outs = np.zeros((N_NODES, HIDDEN), dtype=np.float32)
    for k in range(N_CORES):
        deg, pdeg, nslots = unshard[k]
        arr = np.asarray(res.results[k]["out"])  # [128, npairs*128]
        # partition p = half*64 + feat ; col = pair*128 + j ; vnode = pair*256 + half*128 + j
        v = (
            arr.reshape(2, 64, npairs, 128)
            .transpose(2, 0, 3, 1)
            .reshape(npairs * 256, 64)
        )
        nvirt = nslots // D
        v = v[:nvirt]
        vstarts = np.concatenate([[0], np.cumsum(pdeg // D)[:-1]])
        nm = np.maximum.reduceat(v, vstarts, axis=0)
        nm[deg == 0] = 0.0
        outs[k * NODES_PER_CORE:(k + 1) * NODES_PER_CORE] = nm
    return outs


# revision 7
# speedup vs baseline: 1.1923x; 1.1923x over previous
"""Trainium2 Bass kernel for nn_ASAP (gnn_message_passing), 8 NeuronCores.

Sharding strategy (edge-parallel, dst-bucketed):
  - Edges are sorted by destination node and bucketed into 8 equal node
    ranges (12500 nodes/core), so each core owns the complete segment-max
    for its node range and no cross-core max reduction is needed.
  - Each core's edge shard is materialized as per-edge records
    [pos_src, pos_dst, x_dst] (the edge-parallel shard of node features),
    shipped in a feature-major layout the TensorEngine consumes directly.
  - Each node's edge list is padded to a multiple of D=4 slots ("virtual
    nodes"), so the per-node max becomes a fixed periodic-4 reduction;
    per-node combining of virtual nodes happens at unshard time.
  - Global scalars (direction norm, per-layer BatchNorm statistics) are
    computed on device and combined with AllReduce collectives.

Device pipeline (single NEFF, SPMD on 8 cores):
  phase A: stream [posS,posD] records, accumulate ||posS-posD||^2 -> AllReduce
  pass 1:  L1 matmul (dir/s fold in weights) + relu evict + stats1 -> AllReduce
  pass 2:  L1 again, L2 matmul (BN1 folded) + bias/pad-kill matmul + relu
           evict + stats2, h2 -> DRAM -> AllReduce
  pass 3:  h2 from DRAM, L3 matmul (BN2 folded) + bias/kill + relu evict +
           stats3 + periodic-4 max -> vnode buffer -> AllReduce
  pass 4:  BN3 affine (monotone, commutes with max) applied to vnode maxes.
"""

import sys

sys.path.insert(0, "/opt/trn_rl_repo")

import numpy as np
import ml_dtypes

N_NODES = 100000
HIDDEN = 64
IN_CH = 6
BN_EPS = 1e-5
N_CORES = 8
NODES_PER_CORE = N_NODES // N_CORES  # 12500
D = 4          # virtual-node slot granularity
CHUNK = 512    # slots per matmul chunk (free dim)
PAIR = 2 * CHUNK

BF16 = ml_dtypes.bfloat16

_compiled = None  # (nc, npairs) cache


def _ceil_to(x, m):
    return (x + m - 1) // m * m


def _host_shard(x, pos, edge_index):
    """Sort edges by dst, bucket by node range, build per-core slot arrays."""
    src = np.asarray(edge_index[0], dtype=np.int64)
    dst = np.asarray(edge_index[1], dtype=np.int64)
    E = src.shape[0]
    order = np.argsort(dst, kind="stable")
    src_s, dst_s = src[order], dst[order]
    # core boundaries by node range
    bounds = np.searchsorted(dst_s, np.arange(0, N_NODES + 1, NODES_PER_CORE))
    pos32 = np.asarray(pos, dtype=np.float32)
    x32 = np.asarray(x, dtype=np.float32)

    shards = []
    max_slots = 0
    for k in range(N_CORES):
        lo, hi = bounds[k], bounds[k + 1]
        s_k, d_k = src_s[lo:hi], dst_s[lo:hi] - k * NODES_PER_CORE
        deg = np.bincount(d_k, minlength=NODES_PER_CORE)
        pdeg = _ceil_to(deg, D)
        nslots = int(pdeg.sum())
        starts = np.concatenate([[0], np.cumsum(pdeg)[:-1]])
        estarts = np.concatenate([[0], np.cumsum(deg)[:-1]])
        rank = np.arange(hi - lo) - np.repeat(estarts, deg)
        slot = np.repeat(starts, deg) + rank
        shards.append((s_k, d_k, deg, pdeg, nslots, slot))
        max_slots = max(max_slots, nslots)
    S_pad = _ceil_to(max_slots, 2 * PAIR)
    npairs = S_pad // PAIR

    in_maps = []
    unshard = []
    for k in range(N_CORES):
        s_k, d_k, deg, pdeg, nslots, slot = shards[k]
        rec = np.zeros((S_pad, 16), dtype=np.float32)
        rec[slot, 0:3] = pos32[s_k]
        rec[slot, 3:6] = pos32[d_k + k * NODES_PER_CORE]
        rec[slot, 6:9] = x32[d_k + k * NODES_PER_CORE]
        valid = np.zeros(S_pad, dtype=np.float32)
        valid[slot] = 1.0
        # feature-major, two 512-slot chunks stacked on partitions
        feat2 = (
            rec.reshape(npairs, 2, CHUNK, 16)
            .transpose(1, 3, 0, 2)
            .reshape(2 * 16, npairs * CHUNK)
            .astype(BF16)
        )
        # edge-major dir records [S_pad, 8] = [posS(3), posD(3), 0, 0]
        dirrec = np.zeros((S_pad, 8), dtype=np.float32)
        dirrec[:, 0:6] = rec[:, 0:6]
        dirrec = dirrec.astype(BF16)
        # flag rows [4, npairs*CHUNK]: realA, padA, realB, padB
        v2 = valid.reshape(npairs, 2, CHUNK)
        flags = np.zeros((4, npairs, CHUNK), dtype=np.float32)
        flags[0] = v2[:, 0]
        flags[1] = 1.0 - v2[:, 0]
        flags[2] = v2[:, 1]
        flags[3] = 1.0 - v2[:, 1]
        flags = flags.reshape(4, npairs * CHUNK).astype(BF16)
        in_maps.append({"feat2": feat2, "dirrec": dirrec, "flags": flags})
        unshard.append((deg, pdeg, nslots))
    return in_maps, unshard, S_pad, npairs, E


def _build(npairs, E_total):
    import concourse.bacc as bacc
    import concourse.bass as bass
    from concourse import mybir
    from concourse.tile import TileContext

    F32 = mybir.dt.float32
    B16 = mybir.dt.bfloat16
    AX = mybir.AxisListType
    OP = mybir.AluOpType
    ACTF = mybir.ActivationFunctionType
    S2 = npairs * CHUNK  # columns of feat2 (= slots/2)
    NVC = npairs * 128   # vnode columns of output per partition-half

    nc = bacc.Bacc("TRN2", target_bir_lowering=False, debug=False)

    feat2 = nc.declare_dram_parameter("feat2", [32, S2], B16, isOutput=False)
    dirrec = nc.declare_dram_parameter("dirrec", [S2 * 2, 8], B16, isOutput=False)
    flags = nc.declare_dram_parameter("flags", [4, S2], B16, isOutput=False)
    w1dir = nc.declare_dram_parameter("w1dir", [3, HIDDEN], F32, isOutput=False)
    w1x = nc.declare_dram_parameter("w1x", [3, HIDDEN], F32, isOutput=False)
    w2 = nc.declare_dram_parameter("w2", [HIDDEN, HIDDEN], F32, isOutput=False)
    w3 = nc.declare_dram_parameter("w3", [HIDDEN, HIDDEN], F32, isOutput=False)
    # gvec rows: 0:g1 1:be1 2:b1 3:g2 4:be2 5:b2 6:g3 7:be3 8:b3  as [9, 64]
    gvec = nc.declare_dram_parameter("gvec", [9, HIDDEN], F32, isOutput=False)
    out = nc.declare_dram_parameter("out", [128, NVC], F32, isOutput=True)

    h2store = nc.dram_tensor("h2store", [128, S2], B16)
    vnstore = nc.dram_tensor("vnstore", [128, NVC], F32)
    cc_in = nc.dram_tensor("cc_in", [128, 4], F32)
    cc_out = nc.dram_tensor("cc_out", [128, 4], F32, addr_space="Shared")
    RG = [list(range(N_CORES))]

    with TileContext(nc) as tc:
        with (
            tc.tile_pool(name="const", bufs=1) as cp,
            tc.tile_pool(name="stream", bufs=4) as sp,
            tc.tile_pool(name="hbuf", bufs=4) as hp,
            tc.tile_pool(name="acc", bufs=1) as ap,
            tc.tile_pool(name="psum", bufs=2, space="PSUM") as pp,
        ):
            # ---- constants / small tiles ----
            w1d_t = cp.tile([3, HIDDEN], F32)
            nc.sync.dma_start(out=w1d_t[:], in_=w1dir[:])
            w1x_t = cp.tile([3, HIDDEN], F32)
            nc.sync.dma_start(out=w1x_t[:], in_=w1x[:])
            w2_t = cp.tile([HIDDEN, HIDDEN], F32)
            nc.sync.dma_start(out=w2_t[:], in_=w2[:])
            w3_t = cp.tile([HIDDEN, HIDDEN], F32)
            nc.sync.dma_start(out=w3_t[:], in_=w3[:])
            gv_t = cp.tile([9, HIDDEN], F32)
            nc.sync.dma_start(out=gv_t[:], in_=gvec[:])

            # ---- phase A: global direction norm ----
            dacc = ap.tile([128, 1], F32)
            nc.gpsimd.memset(dacc[:], 0.0)
            ntiles_a = (2 * S2) // 4096
            for t in range(ntiles_a):
                dt_ = sp.tile([128, 32, 8], B16, tag="dirt")
                nc.sync.dma_start(
                    out=dt_[:],
                    in_=dirrec[:].rearrange("(a p b) f -> a p b f", p=128, b=32)[t],
                )
                dsub = hp.tile([128, 32, 3], F32, tag="dsub")
                nc.vector.tensor_tensor(
                    out=dsub[:], in0=dt_[:, :, 0:3], in1=dt_[:, :, 3:6], op=OP.subtract
                )
                dsq = hp.tile([128, 32, 3], F32, tag="dsq")
                nc.vector.tensor_tensor(
                    out=dsq[:], in0=dsub[:], in1=dsub[:], op=OP.mult
                )
                dred = hp.tile([128, 1], F32, tag="dred")
                nc.vector.tensor_reduce(
                    out=dred[:], in_=dsq[:].rearrange("p a b -> p (a b)"),
                    axis=AX.X, op=OP.add,
                )
                nc.vector.tensor_tensor(
                    out=dacc[:], in0=dacc[:], in1=dred[:], op=OP.add
                )
            # partition-reduce to scalar then AllReduce
            dtot = ap.tile([1, 1], F32)
            nc.gpsimd.tensor_reduce(
                out=dtot[:], in_=dacc[:], axis=AX.C, op=OP.add
            )
            cc_s = ap.tile([128, 4], F32)
            nc.gpsimd.memset(cc_s[:], 0.0)
            nc.vector.tensor_copy(out=cc_s[0:1, 0:1], in_=dtot[:])
            nc.sync.dma_start(out=cc_in[:], in_=cc_s[:])
            nc.gpsimd.collective_compute(
                "AllReduce", OP.add, replica_groups=RG,
                ins=[cc_in[:]], outs=[cc_out[:]],
            )
            nsq = ap.tile([1, 1], F32)
            nc.sync.dma_start(out=nsq[:], in_=cc_out[0:1, 0:1])
            # rs = 1/||dir|| = reciprocal(sqrt(sum))
            nrm = ap.tile([1, 1], F32)
            nc.scalar.activation(out=nrm[:], in_=nsq[:], func=ACTF.Sqrt)
            rs1 = ap.tile([1, 1], F32)
            nc.vector.reciprocal(out=rs1[:], in_=nrm[:])
            rs128 = ap.tile([128, 1], F32)
            nc.gpsimd.partition_broadcast(rs128[:], rs1[:])

            # ---- build L1 weights: lhsT1 [32, 128] bf16 ----
            lhsT1 = cp.tile([32, 128], B16)
            nc.gpsimd.memset(lhsT1[:], 0.0)
            w1d_p = cp.tile([3, HIDDEN], B16)
            nc.vector.tensor_scalar(
                out=w1d_p[:], in0=w1d_t[:], scalar1=rs128[0:3, 0:1],
                scalar2=None, op0=OP.mult,
            )
            w1d_n = cp.tile([3, HIDDEN], B16)
            nc.vector.tensor_scalar(
                out=w1d_n[:], in0=w1d_t[:], scalar1=rs128[0:3, 0:1],
                scalar2=-1.0, op0=OP.mult, op1=OP.mult,
            )
            w1x_b = cp.tile([3, HIDDEN], B16)
            nc.vector.tensor_copy(out=w1x_b[:], in_=w1x_t[:])
            for ko, base in ((0, 0), (16, 64)):
                nc.sync.dma_start(out=lhsT1[ko + 0:ko + 3, base:base + 64], in_=w1d_p[:])
                nc.sync.dma_start(out=lhsT1[ko + 3:ko + 6, base:base + 64], in_=w1d_n[:])
                nc.sync.dma_start(out=lhsT1[ko + 6:ko + 9, base:base + 64], in_=w1x_b[:])

            # ---- helpers ----
            def stats_sync(stbuf, dst_stats):
                """bn_aggr per-chunk stats, convert to sums, AllReduce, mu/var."""
                agg = ap.tile([128, 2], F32, tag="agg" + dst_stats)
                nc.vector.bn_aggr(out=agg[:], in_=stbuf[:])
                nslots = float(stbuf.shape[1] * CHUNK)
                sred = ap.tile([128, 1], F32, tag="sred" + dst_stats)
                nc.vector.tensor_scalar(
                    out=sred[:], in0=agg[:, 0:1], scalar1=nslots, scalar2=None,
                    op0=OP.mult,
                )
                # sumsq = (var + mean^2) * nslots
                qred = ap.tile([128, 1], F32, tag="qred" + dst_stats)
                nc.vector.tensor_tensor(
                    out=qred[:], in0=agg[:, 0:1], in1=agg[:, 0:1], op=OP.mult
                )
                nc.vector.tensor_tensor(
                    out=qred[:], in0=qred[:], in1=agg[:, 1:2], op=OP.add
                )
                nc.vector.tensor_scalar(
                    out=qred[:], in0=qred[:], scalar1=nslots, scalar2=None,
                    op0=OP.mult,
                )
                cc2 = ap.tile([128, 4], F32, tag="cc2" + dst_stats)
                nc.gpsimd.memset(cc2[:], 0.0)
                hi2 = ap.tile([64, 2], F32, tag="hi2" + dst_stats)
                nc.sync.dma_start(out=hi2[:, 0:1], in_=sred[64:128, :])
                nc.sync.dma_start(out=hi2[:, 1:2], in_=qred[64:128, :])
                nc.vector.tensor_tensor(
                    out=cc2[0:64, 0:1], in0=sred[0:64, :], in1=hi2[:, 0:1], op=OP.add
                )
                nc.vector.tensor_tensor(
                    out=cc2[0:64, 1:2], in0=qred[0:64, :], in1=hi2[:, 1:2], op=OP.add
                )
                nc.sync.dma_start(out=cc_in[:], in_=cc2[:])
                nc.gpsimd.collective_compute(
                    "AllReduce", OP.add, replica_groups=RG,
                    ins=[cc_in[:]], outs=[cc_out[:]],
                )
                gl = ap.tile([64, 2], F32, tag="gl" + dst_stats)
                nc.sync.dma_start(out=gl[:], in_=cc_out[0:64, 0:2])
                invE = 1.0 / float(E_total)
                mu = ap.tile([64, 1], F32, tag="mu" + dst_stats)
                nc.vector.tensor_scalar(
                    out=mu[:], in0=gl[:, 0:1], scalar1=invE, scalar2=None, op0=OP.mult
                )
                musq = ap.tile([64, 1], F32, tag="musq" + dst_stats)
                nc.vector.tensor_tensor(out=musq[:], in0=mu[:], in1=mu[:], op=OP.mult)
                var = ap.tile([64, 1], F32, tag="var" + dst_stats)
                nc.vector.tensor_scalar(
                    out=var[:], in0=gl[:, 1:2], scalar1=invE, scalar2=None, op0=OP.mult
                )
                nc.vector.tensor_tensor(
                    out=var[:], in0=var[:], in1=musq[:], op=OP.subtract
                )
                return mu, var

            def fold_affine(mu, var, g_row, be_row, tag):
                """s = g*rsqrt(var+eps); t = be - mu*s  (per-channel [64,1])."""
                vps = ap.tile([64, 1], F32, tag="vps" + tag)
                nc.vector.tensor_scalar(
                    out=vps[:], in0=var[:], scalar1=BN_EPS, scalar2=None, op0=OP.add
                )
                sd = ap.tile([64, 1], F32, tag="sd" + tag)
                nc.scalar.activation(out=sd[:], in_=vps[:], func=ACTF.Sqrt)
                rsd = ap.tile([64, 1], F32, tag="rsd" + tag)
                nc.vector.reciprocal(out=rsd[:], in_=sd[:])
                # g, be arrive as rows [1,64] of gvec; transpose to col via PE? cheap:
                # use dma transpose-free: gvec rows are [1,64]; we need [64,1].
                # DMA from DRAM with AP [64,1] view of the row.
                gcol = ap.tile([64, 1], F32, tag="gcol" + tag)
                nc.sync.dma_start(out=gcol[:], in_=gvec[g_row, :, None])
                becol = ap.tile([64, 1], F32, tag="becol" + tag)
                nc.sync.dma_start(out=becol[:], in_=gvec[be_row, :, None])
                s_ = ap.tile([64, 1], F32, tag="s" + tag)
                nc.vector.tensor_tensor(out=s_[:], in0=gcol[:], in1=rsd[:], op=OP.mult)
                t_ = ap.tile([64, 1], F32, tag="t" + tag)
                nc.vector.tensor_tensor(out=t_[:], in0=mu[:], in1=s_[:], op=OP.mult)
                nc.vector.tensor_tensor(
                    out=t_[:], in0=becol[:], in1=t_[:], op=OP.subtract
                )
                return s_, t_

            def build_layer(mu, var, g_row, be_row, b_row, w_t, tag):
                """lhsT [128,128] = blockdiag(diag(s)W, diag(s)W) bf16,
                killT [4,128] rows: [beff@A, -BIG@A, beff@B, -BIG@B]."""
                s_, t_ = fold_affine(mu, var, g_row, be_row, tag)
                lhsT = cp.tile([128, 128], B16, tag="lhsT" + tag)
                nc.gpsimd.memset(lhsT[:], 0.0)
                nc.vector.tensor_scalar(
                    out=lhsT[0:64, 0:64], in0=w_t[:], scalar1=s_[:], scalar2=None,
                    op0=OP.mult,
                )
                nc.vector.tensor_scalar(
                    out=lhsT[64:128, 64:128], in0=w_t[:], scalar1=s_[:], scalar2=None,
                    op0=OP.mult,
                )
                # beff row = t^T @ W + b : matmul lhsT=t(col) rhs=W -> [1,64]
                tb = ap.tile([64, 1], B16, tag="tb" + tag)
                nc.vector.tensor_copy(out=tb[:], in_=t_[:])
                wb = ap.tile([64, 64], B16, tag="wb" + tag)
                nc.vector.tensor_copy(out=wb[:], in_=w_t[:])
                bp = pp.tile([1, 64], F32, tag="bp", space="PSUM")
                nc.tensor.matmul(out=bp[:], lhsT=tb[:], rhs=wb[:])
                brow = ap.tile([1, 64], F32, tag="brow" + tag)
                nc.vector.tensor_copy(out=brow[:], in_=bp[:])
                bcol = ap.tile([64, 1], F32, tag="bcol" + tag)
                nc.sync.dma_start(out=bcol[:], in_=gvec[b_row, :, None])
                # add layer bias b (zeros in practice): brow += b^T — b is [64,1]; do
                # via dma-transposed view add
                btr = ap.tile([1, 64], F32, tag="btr" + tag)
                nc.sync.dma_start(out=btr[:], in_=gvec[b_row, None, :])
                nc.vector.tensor_tensor(
                    out=brow[:], in0=brow[:], in1=btr[:], op=OP.add
                )
                killT = cp.tile([4, 128], B16, tag="killT" + tag)
                nc.gpsimd.memset(killT[:], 0.0)
                negbig = ap.tile([1, 64], B16, tag="negbig" + tag)
                nc.gpsimd.memset(negbig[:], -1e30)
                browb = ap.tile([1, 64], B16, tag="browb" + tag)
                nc.vector.tensor_copy(out=browb[:], in_=brow[:])
                nc.sync.dma_start(out=killT[0:1, 0:64], in_=browb[:])
                nc.sync.dma_start(out=killT[2:3, 64:128], in_=browb[:])
                nc.sync.dma_start(out=killT[1:2, 0:64], in_=negbig[:])
                nc.sync.dma_start(out=killT[3:4, 64:128], in_=negbig[:])
                return lhsT, killT, s_, t_

            # ---- pass 1: L1 + stats1 ----
            st1 = ap.tile([128, npairs, 6], F32)
            for c in range(npairs):
                rhs = sp.tile([32, CHUNK], B16, tag="rhs1")
                nc.sync.dma_start(out=rhs[:], in_=feat2[:, c * CHUNK:(c + 1) * CHUNK])
                z1 = pp.tile([128, CHUNK], F32, tag="zA", space="PSUM")
                nc.tensor.matmul(out=z1[:], lhsT=lhsT1[:], rhs=rhs[:])
                h1 = hp.tile([128, CHUNK], B16, tag="h1")
                nc.scalar.activation(
                    out=h1[:], in_=z1[:], func=ACTF.Relu, bias=0.0, scale=1.0
                )
                nc.vector.bn_stats(out=st1[:, c, :], in_=h1[:])
            mu1, var1 = stats_sync(st1, "1")
            lhsT2, killT2, _, _ = build_layer(mu1, var1, 0, 1, 2, w2_t, "2")

            # ---- pass 2: L1 + L2 + stats2, h2 -> DRAM ----
            st2 = ap.tile([128, npairs, 6], F32)
            for c in range(npairs):
                rhs = sp.tile([32, CHUNK], B16, tag="rhs2")
                nc.sync.dma_start(out=rhs[:], in_=feat2[:, c * CHUNK:(c + 1) * CHUNK])
                z1 = pp.tile([128, CHUNK], F32, tag="zA", space="PSUM")
                nc.tensor.matmul(out=z1[:], lhsT=lhsT1[:], rhs=rhs[:])
                h1 = hp.tile([128, CHUNK], B16, tag="h1b")
                nc.scalar.activation(
                    out=h1[:], in_=z1[:], func=ACTF.Relu, bias=0.0, scale=1.0
                )
                fl = sp.tile([4, CHUNK], B16, tag="fl2")
                nc.sync.dma_start(out=fl[:], in_=flags[:, c * CHUNK:(c + 1) * CHUNK])
                z2 = pp.tile([128, CHUNK], F32, tag="zB", space="PSUM")
                nc.tensor.matmul(out=z2[:], lhsT=lhsT2[:], rhs=h1[:], start=True, stop=False)
                nc.tensor.matmul(out=z2[:], lhsT=killT2[:], rhs=fl[:], start=False, stop=True)
                h2 = hp.tile([128, CHUNK], B16, tag="h2")
                nc.scalar.activation(
                    out=h2[:], in_=z2[:], func=ACTF.Relu, bias=0.0, scale=1.0
                )
                nc.scalar.dma_start(
                    out=h2store[:, c * CHUNK:(c + 1) * CHUNK], in_=h2[:]
                )
                nc.vector.bn_stats(out=st2[:, c, :], in_=h2[:])
            mu2, var2 = stats_sync(st2, "2")
            lhsT3, killT3, _, _ = build_layer(mu2, var2, 3, 4, 5, w3_t, "3")

            # ---- pass 3: L3 + stats3 + periodic-4 max ----
            st3 = ap.tile([128, npairs, 6], F32)
            for c in range(npairs):
                h2 = sp.tile([128, CHUNK], B16, tag="h2r")
                nc.sync.dma_start(
                    out=h2[:], in_=h2store[:, c * CHUNK:(c + 1) * CHUNK]
                )
                fl = sp.tile([4, CHUNK], B16, tag="fl3")
                nc.sync.dma_start(out=fl[:], in_=flags[:, c * CHUNK:(c + 1) * CHUNK])
                z3 = pp.tile([128, CHUNK], F32, tag="zB", space="PSUM")
                nc.tensor.matmul(out=z3[:], lhsT=lhsT3[:], rhs=h2[:], start=True, stop=False)
                nc.tensor.matmul(out=z3[:], lhsT=killT3[:], rhs=fl[:], start=False, stop=True)
                h3 = hp.tile([128, CHUNK], B16, tag="h3")
                nc.scalar.activation(
                    out=h3[:], in_=z3[:], func=ACTF.Relu, bias=0.0, scale=1.0
                )
                nc.vector.bn_stats(out=st3[:, c, :], in_=h3[:])
                vmax = hp.tile([128, 128], F32, tag="vmax")
                nc.vector.tensor_reduce(
                    out=vmax[:], in_=h3[:].rearrange("p (a b) -> p a b", b=D),
                    axis=AX.X, op=OP.max,
                )
                nc.scalar.dma_start(
                    out=vnstore[:, c * 128:(c + 1) * 128], in_=vmax[:]
                )
            mu3, var3 = stats_sync(st3, "4")
            s3, t3 = fold_affine(mu3, var3, 6, 7, "5")
            s3d = ap.tile([128, 1], F32)
            nc.sync.dma_start(out=s3d[0:64, :], in_=s3[:])
            nc.sync.dma_start(out=s3d[64:128, :], in_=s3[:])
            t3d = ap.tile([128, 1], F32)
            nc.sync.dma_start(out=t3d[0:64, :], in_=t3[:])
            nc.sync.dma_start(out=t3d[64:128, :], in_=t3[:])

            # ---- pass 4: BN3 affine on vnode maxes ----
            VW = 2048
            for c in range((NVC + VW - 1) // VW):
                w = min(VW, NVC - c * VW)
                vt = sp.tile([128, VW], F32, tag="vt")
                nc.sync.dma_start(
                    out=vt[:, 0:w], in_=vnstore[:, c * VW:c * VW + w]
                )
                ot = hp.tile([128, VW], F32, tag="ot")
                nc.vector.tensor_scalar(
                    out=ot[:, 0:w], in0=vt[:, 0:w], scalar1=s3d[:], scalar2=t3d[:],
                    op0=OP.mult, op1=OP.add,
                )
                nc.sync.dma_start(out=out[:, c * VW:c * VW + w], in_=ot[:, 0:w])

    nc.compile()
    return nc


def kernel(x, pos, edge_index, W1, b1, g1, be1, W2, b2, g2, be2, W3, b3, g3, be3):
    global _compiled
    from concourse.bass_utils import run_bass_kernel_spmd

    in_maps, unshard, S_pad, npairs, E = _host_shard(x, pos, edge_index)

    W1f = np.asarray(W1, dtype=np.float32)
    gv = np.stack([
        np.asarray(g1, dtype=np.float32), np.asarray(be1, dtype=np.float32),
        np.asarray(b1, dtype=np.float32), np.asarray(g2, dtype=np.float32),
        np.asarray(be2, dtype=np.float32), np.asarray(b2, dtype=np.float32),
        np.asarray(g3, dtype=np.float32), np.asarray(be3, dtype=np.float32),
        np.asarray(b3, dtype=np.float32),
    ])
    common = {
        "w1dir": W1f[0:3], "w1x": W1f[3:6],
        "w2": np.asarray(W2, dtype=np.float32),
        "w3": np.asarray(W3, dtype=np.float32),
        "gvec": gv,
    }
    for m in in_maps:
        m.update(common)

    if _compiled is None or _compiled[1] != npairs:
        _compiled = (_build(npairs, E), npairs)
    nc = _compiled[0]

    res = run_bass_kernel_spmd(nc, in_maps, core_ids=list(range(N_CORES)))

    outs = np.zeros((N_NODES, HIDDEN), dtype=np.float32)
    for k in range(N_CORES):
        deg, pdeg, nslots = unshard[k]
        arr = np.asarray(res.results[k]["out"])  # [128, npairs*128]
        # partition p = half*64 + feat ; col = pair*128 + j ; vnode = pair*256 + half*128 + j
        v = (
            arr.reshape(2, 64, npairs, 128)
            .transpose(2, 0, 3, 1)
            .reshape(npairs * 256, 64)
        )
        nvirt = nslots // D
        v = v[:nvirt]
        vstarts = np.concatenate([[0], np.cumsum(pdeg // D)[:-1]])
        nm = np.maximum.reduceat(v, vstarts, axis=0)
        nm[deg == 0] = 0.0
        outs[k * NODES_PER_CORE:(k + 1) * NODES_PER_CORE] = nm
    return outs
